# revision 17
# baseline (speedup 1.0000x reference)
"""Multi-head attention (RoPE, non-causal) forward on 8 TRN2 NeuronCores.

Sharding: tensor-parallel over heads (2 heads/core), zero on-device
collectives. Every core receives the full input activations plus its head
slice of Wq/Wk/Wv/Wo, computes q/k/v projections + RoPE + SDPA + its
row-parallel partial of the output projection, and the host reduces the 8
partial outputs (the row-parallel all-reduce, performed at unshard time).

On-device layouts (per core, bf16 compute):
  xT      [1024 hid, 4096 tok]   tok = b*2048 + t  (host pre-transposed)
  qT/kT   per batch [128 feat, 2048 tok]   feat = hl*64 + d  (2 local heads)
  v       [128 tok-chunk, 2, 64 feats | 64 ones] x32 chunks
  scoresT [128 kpos, 512 q]  in PSUM, exp on ScalarE (scale=1/8 folded in)
  PV      outT[d, q] with ones-augmented V stationary -> row 64 = softmax sum
  out     [1024 emb, 4096 tok]   bf16 partial of y^T (no biases)

The emission order interleaves batch-1 projections into batch-0 SDPA and
batch-0 out-proj into batch-1 SDPA so the TensorEngine never idles longer
than the ~3.4us HAM MID window (it would throttle to 1.2GHz and, because
SDPA has sub-window gaps, never re-warm).

Biases are separable and exact on host: bq/bk are applied on device
(per-partition add at PSUM eviction); bv contributes Wo@bv to y (softmax
rows sum to 1) and bo is additive -- both added during the host reduce.
"""

import functools

import numpy as np
import ml_dtypes

B, S, HID = 2, 2048, 1024
NH, HD = 16, 64
MAX_SEQ = 65536
NCORES = 8
TOK = B * S  # 4096

_BF16 = ml_dtypes.bfloat16


def _build_graph():
    import concourse.bass as bass
    import concourse.mybir as mybir
    import concourse.tile as tile
    from concourse import bacc

    f32 = mybir.dt.float32
    bf16 = mybir.dt.bfloat16

    nc = bacc.Bacc(
        "TRN2", target_bir_lowering=False, debug=False, num_devices=NCORES
    )

    xT = nc.dram_tensor("xT", [HID, TOK], bf16, kind="ExternalInput")
    wqT = nc.dram_tensor("wqT", [HID, 128], bf16, kind="ExternalInput")
    wkT = nc.dram_tensor("wkT", [HID, 128], bf16, kind="ExternalInput")
    wvT = nc.dram_tensor("wvT", [HID, 128], bf16, kind="ExternalInput")
    woT = nc.dram_tensor("woT", [128, HID], bf16, kind="ExternalInput")
    bqk = nc.dram_tensor("bqk", [128, 2], f32, kind="ExternalInput")
    cosT = nc.dram_tensor("cosT", [128, S], bf16, kind="ExternalInput")
    sinT = nc.dram_tensor("sinT", [128, S], bf16, kind="ExternalInput")
    rT = nc.dram_tensor("rT", [128, 128], bf16, kind="ExternalInput")
    outp = nc.dram_tensor("out", [HID, TOK], bf16, kind="ExternalOutput")

    Exp = mybir.ActivationFunctionType.Exp

    with tile.TileContext(nc, pool_alloc_mode="queue") as tc:
        with (
            tc.tile_pool(name="const", bufs=1) as const,
            tc.tile_pool(name="persist", bufs=1) as persist,
        ):
            # ---- persistent SBUF state ----
            # critical-path DMAs first: stripe-0 activations + q/k weights.
            # Chunked so the first accumulation matmul starts after ~128KB.
            wq_sb = const.tile([128, 8, 128], bf16)
            wk_sb = const.tile([128, 8, 128], bf16)
            wv_sb = const.tile([128, 8, 128], bf16)
            for w_sb, w_dram in ((wq_sb, wqT), (wk_sb, wkT)):
                nc.sync.dma_start(
                    out=w_sb,
                    in_=bass.AP(
                        tensor=w_dram.ap().tensor,
                        offset=0,
                        ap=[[128, 128], [128 * 128, 8], [1, 128]],
                    ),
                )
            xs0_c = [
                const.tile([128, 512], bf16, tag=f"xs0_{k}", name=f"xs0_{k}")
                for k in range(8)
            ]
            for k in range(8):
                nc.sync.dma_start(
                    out=xs0_c[k],
                    in_=bass.AP(
                        tensor=xT.ap().tensor,
                        offset=k * 128 * TOK,
                        ap=[[TOK, 128], [1, 512]],
                    ),
                )
            cos_sb = const.tile([128, S], bf16)
            nc.sync.dma_start(out=cos_sb, in_=cosT.ap())
            sin_sb = const.tile([128, S], bf16)
            nc.sync.dma_start(out=sin_sb, in_=sinT.ap())
            rT_sb = const.tile([128, 128], bf16)
            nc.sync.dma_start(out=rT_sb, in_=rT.ap())
            bqk_sb = const.tile([128, 2], f32)
            nc.sync.dma_start(out=bqk_sb, in_=bqk.ap())

            nc.sync.dma_start(
                out=wv_sb,
                in_=bass.AP(
                    tensor=wvT.ap().tensor,
                    offset=0,
                    ap=[[128, 128], [128 * 128, 8], [1, 128]],
                ),
            )
            wo_sb = const.tile([128, HID], bf16)
            nc.sync.dma_start(out=wo_sb, in_=woT.ap())

            qT_b = [
                persist.tile([128, S], bf16, tag=f"qT{b}", name=f"qT{b}")
                for b in range(2)
            ]
            kT_b = [
                persist.tile([128, S], bf16, tag=f"kT{b}", name=f"kT{b}")
                for b in range(2)
            ]
            outT_q = [
                [
                    persist.tile(
                        [128, 512], bf16, tag=f"oT{b}_{q}", name=f"oT{b}_{q}"
                    )
                    for q in range(4)
                ]
                for b in range(2)
            ]
            # per 128-token chunk: [tok, head, 64 feats | 64 ones]
            vt = [
                persist.tile([128, 2, 128], bf16, tag=f"vt{i}", name=f"vt{i}")
                for i in range(32)
            ]
            for i in range(32):
                nc.gpsimd.memset(vt[i][:, :, 64:128], 1.0)

            with (
                tc.tile_pool(name="scps", bufs=2, space="PSUM") as scps,
                tc.tile_pool(name="pvps", bufs=1, space="PSUM") as pvps,
                tc.tile_pool(name="probs", bufs=4) as probs_pool,
                tc.tile_pool(name="norm", bufs=3) as norm_pool,
            ):

                def sdpa_group(b, hl, qs):
                    hs = slice(hl * 64, (hl + 1) * 64)
                    qcol = qs * 512
                    pv = pvps.tile([128, 512], f32, tag="pv", name="pv")

                    def pv_mms(pr, sg):
                        for i2 in range(2):
                            kc = sg * 2 + i2
                            nc.tensor.matmul(
                                pv,
                                lhsT=vt[b * 16 + kc][:, hl, :],
                                rhs=pr[:, i2 * 512 : (i2 + 1) * 512],
                                start=(kc == 0),
                                stop=(kc == 15),
                            )

                    # PV trails the score-group pipeline by one step so the
                    # PE works on scores sg+1 while ScalarE exponentiates sg.
                    prev = None
                    for sg in range(8):
                        sc = scps.tile([128, 1024], f32, tag="sc", name="sc")
                        for i2 in range(2):
                            kcol = (sg * 2 + i2) * 128
                            nc.tensor.matmul(
                                sc[:, i2 * 512 : (i2 + 1) * 512],
                                lhsT=kT_b[b][hs, kcol : kcol + 128],
                                rhs=qT_b[b][hs, qcol : qcol + 512],
                                start=True,
                                stop=True,
                            )
                        pr = probs_pool.tile(
                            [128, 1024], bf16, tag="pr", name="pr"
                        )
                        nc.scalar.activation(pr, sc, Exp, scale=0.125)
                        if prev is not None:
                            pv_mms(prev, sg - 1)
                        prev = pr
                    pv_mms(prev, 7)
                    srow = norm_pool.tile([1, 512], f32, tag="srow", name="srow")
                    nc.vector.tensor_copy(srow, pv[64:65, :])
                    rec = norm_pool.tile([1, 512], f32, tag="rec", name="rec")
                    nc.vector.reciprocal_approx_fast(rec, srow)
                    bc_sb = norm_pool.tile([64, 512], f32, tag="bcs", name="bcs")
                    nc.gpsimd.partition_broadcast(bc_sb, rec)
                    nc.vector.tensor_mul(
                        outT_q[b][qs][hs, :], pv[0:64, :], bc_sb
                    )

                # ---- projections + RoPE (8 token stripes of 512) and SDPA,
                # interleaved so the PE never idles across the transition ----
                with (
                    tc.tile_pool(name="xpool", bufs=2) as xpool,
                    tc.tile_pool(name="qkps", bufs=2, space="PSUM") as qkps,
                    tc.tile_pool(name="vps", bufs=1, space="PSUM") as vps,
                    tc.tile_pool(name="pre", bufs=3) as pre,
                ):

                    # HAM warm-up: ~3.5us of gap-free dummy matmuls on the
                    # first-arriving weight tile so the PE un-throttles to
                    # 2.4GHz before (and while) the x chunks land.
                    warm_ps = qkps.tile([128, 512], f32, tag="qk", name="warm")
                    for wi in range(24):
                        nc.tensor.matmul(
                            warm_ps,
                            lhsT=wq_sb[:, 0, :],
                            rhs=wq_sb[:, 0:4, :],
                            start=(wi == 0),
                            stop=(wi == 23),
                        )

                    def stripe(s):
                        sb_, sl = divmod(s, 4)
                        if s == 0:
                            xc = lambda kc: xs0_c[kc][:, :]
                        else:
                            xs = xpool.tile(
                                [128, 8, 512], bf16, tag="x", name="xs"
                            )
                            nc.sync.dma_start(
                                out=xs,
                                in_=bass.AP(
                                    tensor=xT.ap().tensor,
                                    offset=s * 512,
                                    ap=[[TOK, 128], [TOK * 128, 8], [1, 512]],
                                ),
                            )
                            xc = lambda kc: xs[:, kc, :]
                        pcol = sl * 512
                        for w_sb, bias_col, dest in (
                            (wq_sb, 0, qT_b[sb_]),
                            (wk_sb, 1, kT_b[sb_]),
                        ):
                            ps = qkps.tile([128, 512], f32, tag="qk", name="ps")
                            for kc in range(8):
                                nc.tensor.matmul(
                                    ps,
                                    lhsT=w_sb[:, kc, :],
                                    rhs=xc(kc),
                                    start=(kc == 0),
                                    stop=(kc == 7),
                                )
                            pre_sb = pre.tile(
                                [128, 512], bf16, tag="pre", name="pre_sb"
                            )
                            nc.vector.tensor_scalar_add(
                                pre_sb, ps, bqk_sb[:, bias_col : bias_col + 1]
                            )
                            rq = qkps.tile([128, 512], f32, tag="qk", name="rq")
                            nc.tensor.matmul(
                                rq, lhsT=rT_sb, rhs=pre_sb, start=True, stop=True
                            )
                            t1 = pre.tile([128, 512], f32, tag="t1", name="t1")
                            nc.vector.tensor_mul(
                                t1, pre_sb, cos_sb[:, pcol : pcol + 512]
                            )
                            t2 = pre.tile([128, 512], f32, tag="t2", name="t2")
                            nc.vector.tensor_mul(
                                t2, rq, sin_sb[:, pcol : pcol + 512]
                            )
                            nc.vector.tensor_add(
                                dest[:, pcol : pcol + 512], t1, t2
                            )
                        for t4 in range(4):
                            vp = vps.tile([128, 128], f32, tag="v", name="vp")
                            for kc in range(8):
                                nc.tensor.matmul(
                                    vp,
                                    lhsT=xc(kc)[:, t4 * 128 : (t4 + 1) * 128],
                                    rhs=wv_sb[:, kc, :],
                                    start=(kc == 0),
                                    stop=(kc == 7),
                                )
                            nc.vector.tensor_copy(
                                vt[s * 4 + t4][:, :, 0:64],
                                vp[:, :].rearrange("p (a d) -> p a d", a=2),
                            )

                    for s in range(4):
                        stripe(s)
                    g = 0
                    for hl in range(2):
                        for qs in range(4):
                            sdpa_group(0, hl, qs)
                            if 1 <= g <= 4:
                                stripe(3 + g)
                            g += 1

                # ---- batch-1 SDPA interleaved with out-proj columns ----
                with (
                    tc.tile_pool(name="yps", bufs=2, space="PSUM") as yps,
                    tc.tile_pool(name="ysb", bufs=2) as ysb_pool,
                ):

                    def outproj_unit(b, n, alternate):
                        # one 512-token column, all 8 embed chunks
                        yb = ysb_pool.tile(
                            [128, 8, 512], bf16, tag="yb", name="yb"
                        )
                        for e in range(8):
                            yp = yps.tile([128, 512], f32, tag="y", name="yp")
                            nc.tensor.matmul(
                                yp,
                                lhsT=wo_sb[:, e * 128 : (e + 1) * 128],
                                rhs=outT_q[b][n][:, :],
                                start=True,
                                stop=True,
                            )
                            if alternate and e % 2 == 1:
                                nc.scalar.copy(yb[:, e, :], yp)
                            else:
                                nc.vector.tensor_copy(yb[:, e, :], yp)
                        nc.sync.dma_start(
                            out=bass.AP(
                                tensor=outp.ap().tensor,
                                offset=b * S + n * 512,
                                ap=[[TOK, 128], [TOK * 128, 8], [1, 512]],
                            ),
                            in_=yb,
                        )

                    for qs in range(4):
                        sdpa_group(1, 0, qs)
                        if qs >= 1:
                            outproj_unit(0, qs - 1, alternate=False)
                        sdpa_group(1, 1, qs)
                        if qs >= 1:
                            outproj_unit(1, qs - 1, alternate=False)
                    outproj_unit(1, 3, alternate=True)
                    outproj_unit(0, 3, alternate=False)

    nc.compile()
    return nc


@functools.lru_cache(maxsize=1)
def _get_graph():
    return _build_graph()


def _rope_tables():
    inv_freq = 1.0 / (
        MAX_SEQ ** (np.arange(0, HD, 2, dtype=np.float32) / HD)
    )
    t = np.arange(S, dtype=np.float32)
    freqs = np.einsum("i,j->ij", t, inv_freq)  # [S, 32]
    emb = np.concatenate([freqs, freqs], axis=-1)  # [S, 64]
    return np.cos(emb), np.sin(emb)


def _rot_matrix():
    r = np.zeros((HD, HD), dtype=np.float32)
    r[np.arange(32), np.arange(32) + 32] = -1.0
    r[np.arange(32) + 32, np.arange(32)] = 1.0
    rt = r.T  # lhsT so that out = R @ q
    return np.block(
        [[rt, np.zeros_like(rt)], [np.zeros_like(rt), rt]]
    )


def make_in_maps(input_embeds, Wq, bq, Wk, bk, Wv, bv, Wo, bo):
    x = np.ascontiguousarray(input_embeds, dtype=np.float32)
    xT = x.reshape(TOK, HID).T.astype(_BF16)  # [1024, 4096]
    cos, sin = _rope_tables()
    cosT = np.tile(cos.T, (2, 1)).astype(_BF16)  # [128, 2048]
    sinT = np.tile(sin.T, (2, 1)).astype(_BF16)
    rT = _rot_matrix().astype(_BF16)
    WqT = Wq.T.astype(_BF16)  # [hid, feat]
    WkT = Wk.T.astype(_BF16)
    WvT = Wv.T.astype(_BF16)
    WoT = Wo.T.astype(_BF16)  # [feat, emb]
    in_maps = []
    for c in range(NCORES):
        fs = slice(c * 128, (c + 1) * 128)
        in_maps.append(
            {
                "xT": xT,
                "wqT": np.ascontiguousarray(WqT[:, fs]),
                "wkT": np.ascontiguousarray(WkT[:, fs]),
                "wvT": np.ascontiguousarray(WvT[:, fs]),
                "woT": np.ascontiguousarray(WoT[fs, :]),
                "bqk": np.ascontiguousarray(
                    np.stack([bq[fs], bk[fs]], axis=1).astype(np.float32)
                ),
                "cosT": cosT,
                "sinT": sinT,
                "rT": rT,
            }
        )
    return in_maps


def reduce_outputs(results, Wq, bq, Wk, bk, Wv, bv, Wo, bo):
    acc = np.zeros((HID, TOK), dtype=np.float32)
    for c in range(NCORES):
        acc += results[c]["out"].astype(np.float32)
    bias = bo.astype(np.float32) + Wo.astype(np.float32) @ bv.astype(np.float32)
    acc += bias[:, None]
    return np.ascontiguousarray(acc.T).reshape(B, S, HID)


def kernel(input_embeds, Wq, bq, Wk, bk, Wv, bv, Wo, bo):
    from concourse.bass_utils import run_bass_kernel_spmd

    nc = _get_graph()
    in_maps = make_in_maps(input_embeds, Wq, bq, Wk, bk, Wv, bv, Wo, bo)
    res = run_bass_kernel_spmd(
        nc, in_maps, core_ids=list(range(NCORES))
    )
    return reduce_outputs(res.results, Wq, bq, Wk, bk, Wv, bv, Wo, bo)


# revision 18
# speedup vs baseline: 1.0312x; 1.0312x over previous
"""Multi-head attention (RoPE, non-causal) forward on 8 TRN2 NeuronCores.

Sharding: tensor-parallel over heads (2 heads/core), zero on-device
collectives. Every core receives the full input activations plus its head
slice of Wq/Wk/Wv/Wo, computes q/k/v projections + RoPE + SDPA + its
row-parallel partial of the output projection, and the host reduces the 8
partial outputs (the row-parallel all-reduce, performed at unshard time).

On-device layouts (per core, bf16 compute):
  xT      [1024 hid, 4096 tok]   tok = b*2048 + t  (host pre-transposed)
  qT/kT   per batch [128 feat, 2048 tok]   feat = hl*64 + d  (2 local heads)
  v       [128 tok-chunk, 2, 64 feats | 64 ones] x32 chunks
  scoresT [128 kpos, 512 q]  in PSUM, exp on ScalarE (scale=1/8 folded in)
  PV      outT[d, q] with ones-augmented V stationary -> row 64 = softmax sum
  out     [1024 emb, 4096 tok]   bf16 partial of y^T (no biases)

The emission order interleaves batch-1 projections into batch-0 SDPA and
batch-0 out-proj into batch-1 SDPA so the TensorEngine never idles longer
than the ~3.4us HAM MID window (it would throttle to 1.2GHz and, because
SDPA has sub-window gaps, never re-warm).

Biases are separable and exact on host: bq/bk are applied on device
(per-partition add at PSUM eviction); bv contributes Wo@bv to y (softmax
rows sum to 1) and bo is additive -- both added during the host reduce.
"""

import functools

import numpy as np
import ml_dtypes

B, S, HID = 2, 2048, 1024
NH, HD = 16, 64
MAX_SEQ = 65536
NCORES = 8
TOK = B * S  # 4096

_BF16 = ml_dtypes.bfloat16


def _build_graph():
    import concourse.bass as bass
    import concourse.mybir as mybir
    import concourse.tile as tile
    from concourse import bacc

    f32 = mybir.dt.float32
    bf16 = mybir.dt.bfloat16

    nc = bacc.Bacc(
        "TRN2", target_bir_lowering=False, debug=False, num_devices=NCORES
    )

    xT = nc.dram_tensor("xT", [HID, TOK], bf16, kind="ExternalInput")
    wqT = nc.dram_tensor("wqT", [HID, 128], bf16, kind="ExternalInput")
    wkT = nc.dram_tensor("wkT", [HID, 128], bf16, kind="ExternalInput")
    wvT = nc.dram_tensor("wvT", [HID, 128], bf16, kind="ExternalInput")
    woT = nc.dram_tensor("woT", [128, HID], bf16, kind="ExternalInput")
    bqk = nc.dram_tensor("bqk", [128, 2], f32, kind="ExternalInput")
    cosT = nc.dram_tensor("cosT", [128, S], bf16, kind="ExternalInput")
    sinT = nc.dram_tensor("sinT", [128, S], bf16, kind="ExternalInput")
    rT = nc.dram_tensor("rT", [128, 128], bf16, kind="ExternalInput")
    outp = nc.dram_tensor("out", [HID, TOK], bf16, kind="ExternalOutput")

    Exp = mybir.ActivationFunctionType.Exp

    with tile.TileContext(nc, pool_alloc_mode="queue") as tc:
        with (
            tc.tile_pool(name="const", bufs=1) as const,
            tc.tile_pool(name="persist", bufs=1) as persist,
        ):
            # ---- persistent SBUF state ----
            # critical-path DMAs first: stripe-0 activations + q/k weights.
            # Chunked so the first accumulation matmul starts after ~128KB.
            wq_sb = const.tile([128, 8, 128], bf16)
            wk_sb = const.tile([128, 8, 128], bf16)
            wv_sb = const.tile([128, 8, 128], bf16)
            for w_sb, w_dram in ((wq_sb, wqT), (wk_sb, wkT)):
                nc.sync.dma_start(
                    out=w_sb,
                    in_=bass.AP(
                        tensor=w_dram.ap().tensor,
                        offset=0,
                        ap=[[128, 128], [128 * 128, 8], [1, 128]],
                    ),
                )
            xs0_c = [
                const.tile([128, 512], bf16, tag=f"xs0_{k}", name=f"xs0_{k}")
                for k in range(8)
            ]
            for k in range(8):
                nc.sync.dma_start(
                    out=xs0_c[k],
                    in_=bass.AP(
                        tensor=xT.ap().tensor,
                        offset=k * 128 * TOK,
                        ap=[[TOK, 128], [1, 512]],
                    ),
                )
            cos_sb = const.tile([128, S], bf16)
            nc.sync.dma_start(out=cos_sb, in_=cosT.ap())
            sin_sb = const.tile([128, S], bf16)
            nc.sync.dma_start(out=sin_sb, in_=sinT.ap())
            rT_sb = const.tile([128, 128], bf16)
            nc.sync.dma_start(out=rT_sb, in_=rT.ap())
            bqk_sb = const.tile([128, 2], f32)
            nc.sync.dma_start(out=bqk_sb, in_=bqk.ap())

            nc.sync.dma_start(
                out=wv_sb,
                in_=bass.AP(
                    tensor=wvT.ap().tensor,
                    offset=0,
                    ap=[[128, 128], [128 * 128, 8], [1, 128]],
                ),
            )
            wo_sb = const.tile([128, HID], bf16)
            nc.sync.dma_start(out=wo_sb, in_=woT.ap())

            qT_b = [
                persist.tile([128, S], bf16, tag=f"qT{b}", name=f"qT{b}")
                for b in range(2)
            ]
            kT_b = [
                persist.tile([128, S], bf16, tag=f"kT{b}", name=f"kT{b}")
                for b in range(2)
            ]
            outT_q = [
                [
                    persist.tile(
                        [128, 512], bf16, tag=f"oT{b}_{q}", name=f"oT{b}_{q}"
                    )
                    for q in range(4)
                ]
                for b in range(2)
            ]
            # per 128-token chunk: [tok, head, 64 feats | 64 ones]
            vt = [
                persist.tile([128, 2, 128], bf16, tag=f"vt{i}", name=f"vt{i}")
                for i in range(32)
            ]
            for i in range(32):
                nc.gpsimd.memset(vt[i][:, :, 64:128], 1.0)

            with (
                tc.tile_pool(name="scps", bufs=2, space="PSUM") as scps,
                tc.tile_pool(name="pvps", bufs=1, space="PSUM") as pvps,
                tc.tile_pool(name="probs", bufs=4) as probs_pool,
                tc.tile_pool(name="norm", bufs=3) as norm_pool,
            ):

                def sdpa_group(b, hl, qs):
                    hs = slice(hl * 64, (hl + 1) * 64)
                    qcol = qs * 512
                    pv = pvps.tile([128, 512], f32, tag="pv", name="pv")

                    def pv_mms(pr, sg):
                        for i2 in range(2):
                            kc = sg * 2 + i2
                            nc.tensor.matmul(
                                pv,
                                lhsT=vt[b * 16 + kc][:, hl, :],
                                rhs=pr[:, i2 * 512 : (i2 + 1) * 512],
                                start=(kc == 0),
                                stop=(kc == 15),
                            )

                    # PV trails the score-group pipeline by one step so the
                    # PE works on scores sg+1 while ScalarE exponentiates sg.
                    prev = None
                    for sg in range(8):
                        sc = scps.tile([128, 1024], f32, tag="sc", name="sc")
                        for i2 in range(2):
                            kcol = (sg * 2 + i2) * 128
                            nc.tensor.matmul(
                                sc[:, i2 * 512 : (i2 + 1) * 512],
                                lhsT=kT_b[b][hs, kcol : kcol + 128],
                                rhs=qT_b[b][hs, qcol : qcol + 512],
                                start=True,
                                stop=True,
                            )
                        pr = probs_pool.tile(
                            [128, 1024], bf16, tag="pr", name="pr"
                        )
                        nc.scalar.activation(pr, sc, Exp, scale=0.125)
                        if prev is not None:
                            pv_mms(prev, sg - 1)
                        prev = pr
                    pv_mms(prev, 7)
                    srow = norm_pool.tile([1, 512], f32, tag="srow", name="srow")
                    nc.vector.tensor_copy(srow, pv[64:65, :])
                    rec = norm_pool.tile([1, 512], f32, tag="rec", name="rec")
                    nc.vector.reciprocal_approx_fast(rec, srow)
                    bc_sb = norm_pool.tile([64, 512], f32, tag="bcs", name="bcs")
                    nc.gpsimd.partition_broadcast(bc_sb, rec)
                    nc.vector.tensor_mul(
                        outT_q[b][qs][hs, :], pv[0:64, :], bc_sb
                    )

                # ---- projections + RoPE (8 token stripes of 512) and SDPA,
                # interleaved so the PE never idles across the transition ----
                with (
                    tc.tile_pool(name="xpool", bufs=2) as xpool,
                    tc.tile_pool(name="qkps", bufs=2, space="PSUM") as qkps,
                    tc.tile_pool(name="vps", bufs=1, space="PSUM") as vps,
                    tc.tile_pool(name="pre", bufs=3) as pre,
                ):

                    # HAM warm-up: ~3.5us of gap-free dummy matmuls on the
                    # first-arriving weight tile so the PE un-throttles to
                    # 2.4GHz before (and while) the x chunks land.
                    warm_ps = qkps.tile([128, 512], f32, tag="qk", name="warm")
                    for wi in range(10):
                        nc.tensor.matmul(
                            warm_ps,
                            lhsT=wq_sb[:, 0, :],
                            rhs=wq_sb[:, 0:4, :],
                            start=(wi == 0),
                            stop=(wi == 9),
                        )

                    def stripe(s):
                        sb_, sl = divmod(s, 4)
                        if s == 0:
                            xc = lambda kc: xs0_c[kc][:, :]
                        else:
                            xs = xpool.tile(
                                [128, 8, 512], bf16, tag="x", name="xs"
                            )
                            nc.sync.dma_start(
                                out=xs,
                                in_=bass.AP(
                                    tensor=xT.ap().tensor,
                                    offset=s * 512,
                                    ap=[[TOK, 128], [TOK * 128, 8], [1, 512]],
                                ),
                            )
                            xc = lambda kc: xs[:, kc, :]
                        pcol = sl * 512
                        for w_sb, bias_col, dest in (
                            (wq_sb, 0, qT_b[sb_]),
                            (wk_sb, 1, kT_b[sb_]),
                        ):
                            ps = qkps.tile([128, 512], f32, tag="qk", name="ps")
                            for kc in range(8):
                                nc.tensor.matmul(
                                    ps,
                                    lhsT=w_sb[:, kc, :],
                                    rhs=xc(kc),
                                    start=(kc == 0),
                                    stop=(kc == 7),
                                )
                            pre_sb = pre.tile(
                                [128, 512], bf16, tag="pre", name="pre_sb"
                            )
                            nc.vector.tensor_scalar_add(
                                pre_sb, ps, bqk_sb[:, bias_col : bias_col + 1]
                            )
                            rq = qkps.tile([128, 512], f32, tag="qk", name="rq")
                            nc.tensor.matmul(
                                rq, lhsT=rT_sb, rhs=pre_sb, start=True, stop=True
                            )
                            t1 = pre.tile([128, 512], f32, tag="t1", name="t1")
                            nc.vector.tensor_mul(
                                t1, pre_sb, cos_sb[:, pcol : pcol + 512]
                            )
                            t2 = pre.tile([128, 512], f32, tag="t2", name="t2")
                            nc.vector.tensor_mul(
                                t2, rq, sin_sb[:, pcol : pcol + 512]
                            )
                            nc.vector.tensor_add(
                                dest[:, pcol : pcol + 512], t1, t2
                            )
                        for t4 in range(4):
                            vp = vps.tile([128, 128], f32, tag="v", name="vp")
                            for kc in range(8):
                                nc.tensor.matmul(
                                    vp,
                                    lhsT=xc(kc)[:, t4 * 128 : (t4 + 1) * 128],
                                    rhs=wv_sb[:, kc, :],
                                    start=(kc == 0),
                                    stop=(kc == 7),
                                )
                            nc.vector.tensor_copy(
                                vt[s * 4 + t4][:, :, 0:64],
                                vp[:, :].rearrange("p (a d) -> p a d", a=2),
                            )

                    for s in range(4):
                        stripe(s)
                    g = 0
                    for hl in range(2):
                        for qs in range(4):
                            sdpa_group(0, hl, qs)
                            if 1 <= g <= 4:
                                stripe(3 + g)
                            g += 1

                # ---- batch-1 SDPA interleaved with out-proj columns ----
                with (
                    tc.tile_pool(name="yps", bufs=2, space="PSUM") as yps,
                    tc.tile_pool(name="ysb", bufs=2) as ysb_pool,
                ):

                    def outproj_unit(b, n, alternate):
                        # one 512-token column, all 8 embed chunks
                        yb = ysb_pool.tile(
                            [128, 8, 512], bf16, tag="yb", name="yb"
                        )
                        for e in range(8):
                            yp = yps.tile([128, 512], f32, tag="y", name="yp")
                            nc.tensor.matmul(
                                yp,
                                lhsT=wo_sb[:, e * 128 : (e + 1) * 128],
                                rhs=outT_q[b][n][:, :],
                                start=True,
                                stop=True,
                            )
                            if alternate and e % 2 == 1:
                                nc.scalar.copy(yb[:, e, :], yp)
                            else:
                                nc.vector.tensor_copy(yb[:, e, :], yp)
                        nc.sync.dma_start(
                            out=bass.AP(
                                tensor=outp.ap().tensor,
                                offset=b * S + n * 512,
                                ap=[[TOK, 128], [TOK * 128, 8], [1, 512]],
                            ),
                            in_=yb,
                        )

                    for qs in range(4):
                        sdpa_group(1, 0, qs)
                        if qs >= 1:
                            outproj_unit(0, qs - 1, alternate=False)
                        sdpa_group(1, 1, qs)
                        if qs >= 1:
                            outproj_unit(1, qs - 1, alternate=False)
                    outproj_unit(1, 3, alternate=True)
                    outproj_unit(0, 3, alternate=False)

    nc.compile()
    return nc


@functools.lru_cache(maxsize=1)
def _get_graph():
    return _build_graph()


def _rope_tables():
    inv_freq = 1.0 / (
        MAX_SEQ ** (np.arange(0, HD, 2, dtype=np.float32) / HD)
    )
    t = np.arange(S, dtype=np.float32)
    freqs = np.einsum("i,j->ij", t, inv_freq)  # [S, 32]
    emb = np.concatenate([freqs, freqs], axis=-1)  # [S, 64]
    return np.cos(emb), np.sin(emb)


def _rot_matrix():
    r = np.zeros((HD, HD), dtype=np.float32)
    r[np.arange(32), np.arange(32) + 32] = -1.0
    r[np.arange(32) + 32, np.arange(32)] = 1.0
    rt = r.T  # lhsT so that out = R @ q
    return np.block(
        [[rt, np.zeros_like(rt)], [np.zeros_like(rt), rt]]
    )


def make_in_maps(input_embeds, Wq, bq, Wk, bk, Wv, bv, Wo, bo):
    x = np.ascontiguousarray(input_embeds, dtype=np.float32)
    xT = x.reshape(TOK, HID).T.astype(_BF16)  # [1024, 4096]
    cos, sin = _rope_tables()
    cosT = np.tile(cos.T, (2, 1)).astype(_BF16)  # [128, 2048]
    sinT = np.tile(sin.T, (2, 1)).astype(_BF16)
    rT = _rot_matrix().astype(_BF16)
    WqT = Wq.T.astype(_BF16)  # [hid, feat]
    WkT = Wk.T.astype(_BF16)
    WvT = Wv.T.astype(_BF16)
    WoT = Wo.T.astype(_BF16)  # [feat, emb]
    in_maps = []
    for c in range(NCORES):
        fs = slice(c * 128, (c + 1) * 128)
        in_maps.append(
            {
                "xT": xT,
                "wqT": np.ascontiguousarray(WqT[:, fs]),
                "wkT": np.ascontiguousarray(WkT[:, fs]),
                "wvT": np.ascontiguousarray(WvT[:, fs]),
                "woT": np.ascontiguousarray(WoT[fs, :]),
                "bqk": np.ascontiguousarray(
                    np.stack([bq[fs], bk[fs]], axis=1).astype(np.float32)
                ),
                "cosT": cosT,
                "sinT": sinT,
                "rT": rT,
            }
        )
    return in_maps


def reduce_outputs(results, Wq, bq, Wk, bk, Wv, bv, Wo, bo):
    acc = np.zeros((HID, TOK), dtype=np.float32)
    for c in range(NCORES):
        acc += results[c]["out"].astype(np.float32)
    bias = bo.astype(np.float32) + Wo.astype(np.float32) @ bv.astype(np.float32)
    acc += bias[:, None]
    return np.ascontiguousarray(acc.T).reshape(B, S, HID)


def kernel(input_embeds, Wq, bq, Wk, bk, Wv, bv, Wo, bo):
    from concourse.bass_utils import run_bass_kernel_spmd

    nc = _get_graph()
    in_maps = make_in_maps(input_embeds, Wq, bq, Wk, bk, Wv, bv, Wo, bo)
    res = run_bass_kernel_spmd(
        nc, in_maps, core_ids=list(range(NCORES))
    )
    return reduce_outputs(res.results, Wq, bq, Wk, bk, Wv, bv, Wo, bo)
